# revision 5
# baseline (speedup 1.0000x reference)
"""Trainium2 Bass kernel for CharacterNet segment-mean + FC (segment_reduce).

Reference computation (per batch row b of 32):
  x = all_encoder_layers[layer_index][b]          # (512, 768)
  for t in 0..255: mean_t = mean(x[token_map[b,t]:token_map[b,t+1]])
  ote[b*256+t] = mean_t                           # (8192, 768) output 2
  rep = ote @ fc_w.T + fc_b                       # (8192, 768) output 1

Strategy: data-parallel over batch across 8 NeuronCores (4 rows/core).
All HBM I/O is 16-bit or fp8 (rel-err budget 2e-2 allows it; measured
~4e-3): x/outputs bf16, fc_w as two fp8 tensors (hi + 16x-scaled lo
residual).  The segment mean is a matmul with a one-hot selection
matrix Sel (s, t) built on device; stage 1 computes ote directly in
natural orientation (t on partitions), PE-transposes feed stage 2
(the FC) which runs as fp8 DoubleRow matmuls at 0.5 cyc/row using the
hi/lo split:  rep ~= [m_hi@w_hi + m_hi@w_lo + m_lo@w_hi] / 256, with
m_hi = fp8(16*mean), m_lo = fp8(16*mean - m_hi) (same for w).
"""

import os
import numpy as np
import ml_dtypes

import concourse.bass as bass
import concourse.bacc as bacc
import concourse.mybir as mybir
import concourse.tile as tile
from concourse.bass_utils import run_bass_kernel_spmd  # noqa: F401

N_CORES = 8
B, S, H, T = 32, 512, 768, 256
B_LOC = B // N_CORES          # 4 batch rows per core
NS = S // 128                 # 4 s-chunks per row
NJ = B_LOC * NS               # 16 (128,...) s-chunks per core
NH = H // 128                 # 6 h-chunks

F32 = mybir.dt.float32
BF16 = mybir.dt.bfloat16
FP8 = mybir.dt.float8e4
NPBF16 = ml_dtypes.bfloat16
NPFP8 = ml_dtypes.float8_e4m3

OPT = {
    "s2": os.environ.get("KERNEL_S2", "fp8x2"),  # fp8x2 | bf16
    # NOTE: gpsimd (Pool) cannot access PSUM; psum evicts go DVE/Act only
    "ote_copy": "vector",    # s1 psum->ote sbuf evict engine
    "rep_copy": "scalar",    # s2 psum->rep sbuf evict (scaled copy)
    "mhi_copy": "scalar",    # transpose psum -> m_hi fp8
    "mlo_copy": "vector",    # transpose psum -> m_lo fp8 (stt)
    "mb_copy": "vector",     # bf16-mode transpose evict
    "sel_eng": "gpsimd",
    "in_dma": "sync", "out_dma": "sync",
    "p1": 3, "pt": 2, "p2": 3,
    "w_after": 1,            # emit fc_w DMA after this many x2 pair DMAs
    "x_split_first": True,
}

_ENG = {"vector": "vector", "scalar": "scalar", "gpsimd": "gpsimd"}


def _copy(nc, engine, dst, src_):
    if engine == "scalar":
        nc.scalar.copy(dst, src_)
    elif engine == "gpsimd":
        nc.gpsimd.tensor_copy(dst, src_)
    else:
        nc.vector.tensor_copy(dst, src_)


def _scaled_copy(nc, engine, dst, src_, scale):
    if engine == "scalar":
        nc.scalar.mul(dst, src_, float(scale))
    elif engine == "gpsimd":
        nc.gpsimd.tensor_scalar_mul(dst, src_, float(scale))
    else:
        nc.vector.tensor_scalar_mul(dst, src_, float(scale))


def build_kernel(reps: int = 1, loop: bool = False,
                 bias_mm: bool = False) -> bass.Bass:
    s2_fp8 = OPT["s2"] == "fp8x2"
    nc = bacc.Bacc("TRN2", target_bir_lowering=False, debug=False,
                   num_devices=N_CORES)

    x_d = nc.dram_tensor("x", (NJ * 128, H), BF16, kind="ExternalInput")
    # packed aux: cols 0..15 = seg, 16..31 = inv  (128, 32) f32
    aux_d = nc.dram_tensor("selaux", (128, 2 * NJ), F32, kind="ExternalInput")
    if s2_fp8:
        w_d = nc.dram_tensor("fcw8", (2 * H, H), FP8, kind="ExternalInput")
    else:
        w_d = nc.dram_tensor("fcwb", (H, H), BF16, kind="ExternalInput")
    # packed bias row: [0:H]=fc_b, [H:H+128]=ones  (bf16)
    bias_d = nc.dram_tensor("biasaux", (1, H + 128), BF16,
                            kind="ExternalInput")
    ident_d = nc.dram_tensor("identb", (128, 128), BF16, kind="ExternalInput")
    rep_d = nc.dram_tensor("rep", (B_LOC * T, H), BF16, kind="ExternalOutput")
    ote_d = nc.dram_tensor("ote", (B_LOC * T, H), BF16, kind="ExternalOutput")

    # paired-row-chunk views: [j0][p, q, h] = t[(2*j0+q)*128+p, h]
    x_v = x_d.rearrange("(a q p) h -> a p q h", q=2, p=128)
    rep_v = rep_d.rearrange("(a q p) h -> a p q h", q=2, p=128)
    ote_v = ote_d.rearrange("(a q p) h -> a p q h", q=2, p=128)
    if s2_fp8:
        # (128, 2hl, 6kh, 768n):  w_v[p, l, k, n] = w_d[l*768 + k*128 + p, n]
        w_v = w_d.rearrange("(l k p) n -> p l k n", l=2, p=128)
    else:
        w_v = w_d.rearrange("(k p) n -> p k n", p=128)

    with tile.TileContext(nc) as tc:
        with (
            tc.tile_pool(name="const", bufs=1) as cpool,
            tc.tile_pool(name="xp", bufs=1) as xpool,
            tc.tile_pool(name="selp", bufs=1) as selpool,
            tc.tile_pool(name="mp", bufs=1) as mpool,
            tc.tile_pool(name="wp", bufs=1) as wpool,
            tc.tile_pool(name="ob", bufs=2) as opool,
            tc.tile_pool(name="p1", bufs=OPT["p1"], space="PSUM") as p1pool,
            tc.tile_pool(name="pt", bufs=OPT["pt"], space="PSUM") as ptpool,
            tc.tile_pool(name="p2", bufs=OPT["p2"], space="PSUM") as p2pool,
        ):
            # one-time constants (outside the rep loop)
            iota_t = cpool.tile([128, T], BF16, tag="iota")
            nc.gpsimd.iota(iota_t[:], pattern=[[1, T]], base=0,
                           channel_multiplier=0,
                           allow_small_or_imprecise_dtypes=True)
            ident = cpool.tile([128, 128], BF16, tag="ident")
            nc.sync.dma_start(ident[:], ident_d[:])

            def emit_rep():
                idma = getattr(nc, OPT["in_dma"])
                odma = getattr(nc, OPT["out_dma"])
                aux_sb = cpool.tile([128, 2 * NJ], F32, tag="aux")
                idma.dma_start(aux_sb[:], aux_d[:])
                bias_sb = cpool.tile([1, H + 128], BF16, tag="bias")
                if bias_mm:
                    idma.dma_start(bias_sb[:], bias_d[:])

                # ---- input DMAs + sel builds -------------------------------
                if s2_fp8:
                    w_sb = wpool.tile([128, 2, NH, H], FP8, tag="w")
                else:
                    w_sb = wpool.tile([128, NH, H], BF16, tag="w")
                x2_sb = []

                def emit_w():
                    idma.dma_start(w_sb[:], w_v[:] if not s2_fp8 else w_v)

                for j0 in range(NJ // 2):
                    if j0 == OPT["w_after"]:
                        emit_w()
                    x2 = xpool.tile([128, 2, H], BF16, tag=f"x{j0}")
                    if j0 == 0 and OPT["x_split_first"]:
                        for q in range(2):
                            idma.dma_start(x2[:, q, :],
                                           x_d[q * 128:(q + 1) * 128, :])
                    else:
                        idma.dma_start(x2[:], x_v[j0])
                    x2_sb.append(x2)
                if OPT["w_after"] >= NJ // 2:
                    emit_w()

                sel_eng = getattr(nc, _ENG[OPT["sel_eng"]])
                sel_sb = []
                for j in range(NJ):
                    sel = selpool.tile([128, T], BF16, tag=f"s{j}")
                    # Sel chunk: (s in segment t) * 1/count, bf16
                    sel_eng.tensor_scalar(
                        sel[:], iota_t[:],
                        aux_sb[:, j:j + 1], aux_sb[:, NJ + j:NJ + j + 1],
                        op0=mybir.AluOpType.is_equal,
                        op1=mybir.AluOpType.mult)
                    sel_sb.append(sel)

                # bias broadcast tile (128, H) bf16 via K=1 matmul
                if bias_mm:
                    biasb = cpool.tile([128, H], BF16, tag="biasb")
                    for nh in range(2):
                        pb = p2pool.tile([128, 384], F32, tag="ps2")
                        nc.tensor.matmul(
                            pb[:], bias_sb[:1, H:H + 128],
                            bias_sb[:1, nh * 384:(nh + 1) * 384],
                            start=True, stop=True)
                        _copy(nc, "vector",
                              biasb[:, nh * 384:(nh + 1) * 384], pb[:])

                # ---- per-batch-row stages, software-pipelined on PE --------
                osb = [None] * B_LOC

                def stage1(b):
                    # ote[b] natural layout: psum[t', h] = sum_s Sel[s,t']*x[s,h]
                    osb[b] = opool.tile([128, 2, H], BF16, tag="osb", name="osb")
                    for tq in range(2):
                        for hh in range(2):
                            ps = p1pool.tile([128, 384], F32, tag="ps1")
                            for ks in range(NS):
                                j = b * NS + ks
                                nc.tensor.matmul(
                                    ps[:],
                                    sel_sb[j][:, tq * 128:(tq + 1) * 128],
                                    x2_sb[j // 2][:, j % 2,
                                                  hh * 384:(hh + 1) * 384],
                                    start=(ks == 0), stop=(ks == NS - 1))
                            _copy(nc, OPT["ote_copy"],
                                  osb[b][:, tq, hh * 384:(hh + 1) * 384],
                                  ps[:])
                    odma.dma_start(ote_v[b], osb[b][:])

                mhi = [None] * B_LOC
                mlo = [None] * B_LOC

                def stage_tr(b):
                    # transpose ote[b] -> meanT chunks; evict as fp8 hi/lo
                    # (or single bf16 tile in bf16 mode)
                    if s2_fp8:
                        mhi[b] = mpool.tile([128, NH, T], FP8, tag=f"mh{b}", name=f"mh{b}")
                        mlo[b] = mpool.tile([128, NH, T], FP8, tag=f"ml{b}", name=f"ml{b}")
                    else:
                        mhi[b] = mpool.tile([128, NH, T], BF16, tag=f"mh{b}", name=f"mh{b}")
                    for g in range(3):
                        pt = ptpool.tile([128, 512], BF16, tag="pst")
                        for k in range(4):
                            mh, tq = 2 * g + k // 2, k % 2
                            nc.tensor.transpose(
                                pt[:, k * 128:(k + 1) * 128],
                                osb[b][:, tq, mh * 128:(mh + 1) * 128],
                                ident[:])
                        dhi = mhi[b][:, 2 * g:2 * g + 2, :]
                        if s2_fp8:
                            # m_hi = fp8(16*mean); m_lo = fp8(16*mean - m_hi)
                            _scaled_copy(nc, OPT["mhi_copy"], dhi, pt[:], 16.0)
                            eng = getattr(nc, _ENG[OPT["mlo_copy"]])
                            eng.scalar_tensor_tensor(
                                mlo[b][:, 2 * g:2 * g + 2, :], pt[:], 16.0,
                                dhi,
                                op0=mybir.AluOpType.mult,
                                op1=mybir.AluOpType.subtract)
                        else:
                            _copy(nc, OPT["mb_copy"], dhi, pt[:])

                def stage2(b):
                    rsb = opool.tile([128, 2, H], BF16, tag="rsb")
                    for tq in range(2):
                        for nh in range(2):
                            ps2 = p2pool.tile([128, 384], F32, tag="ps2")
                            nsl = slice(nh * 384, (nh + 1) * 384)
                            tsl = slice(tq * 128, (tq + 1) * 128)
                            if s2_fp8:
                                dr = mybir.MatmulPerfMode.DoubleRow
                                ngrp = 9
                                i = 0
                                for sm, sw in ((0, 0), (0, 1), (1, 0)):
                                    msrc = (mhi, mlo)[sm][b]
                                    for g in range(3):
                                        nc.tensor.matmul(
                                            ps2[:],
                                            msrc[:, 2 * g:2 * g + 2, tsl],
                                            w_sb[:, sw, 2 * g:2 * g + 2, nsl],
                                            start=(i == 0),
                                            stop=(i == ngrp - 1),
                                            perf_mode=dr)
                                        i += 1
                            else:
                                for kh in range(NH):
                                    nc.tensor.matmul(
                                        ps2[:],
                                        mhi[b][:, kh, tsl],
                                        w_sb[:, kh, nsl],
                                        start=(kh == 0), stop=(kh == NH - 1))
                            dst = rsb[:, tq, nsl]
                            scale = (1.0 / 256.0) if s2_fp8 else 1.0
                            if bias_mm:
                                # out = psum*scale + bias
                                nc.vector.scalar_tensor_tensor(
                                    dst, ps2[:], scale, biasb[:, nsl],
                                    op0=mybir.AluOpType.mult,
                                    op1=mybir.AluOpType.add)
                            elif s2_fp8:
                                _scaled_copy(nc, OPT["rep_copy"], dst,
                                             ps2[:], scale)
                            else:
                                _copy(nc, OPT["rep_copy"], dst, ps2[:])
                    odma.dma_start(rep_v[b], rsb[:])

                # PE pipeline: s1(0) s1(1) tr(0) s1(2) s2(0) tr(1) s1(3)
                #              s2(1) tr(2) s2(2) tr(3) s2(3)
                stage1(0)
                stage1(1)
                stage_tr(0)
                stage1(2)
                stage2(0)
                stage_tr(1)
                stage1(3)
                stage2(1)
                stage_tr(2)
                stage2(2)
                stage_tr(3)
                stage2(3)

            if loop and reps > 1:
                with tc.For_i(0, reps, 1,
                              hint_engines=(mybir.EngineType.PE,)):
                    emit_rep()
            else:
                for _ in range(reps):
                    emit_rep()

    nc.compile()
    return nc


def _host_prep(all_encoder_layers, token_map, fc_w, fc_b, layer_index):
    """Slice the chosen layer and build per-core input maps."""
    layer = int(np.asarray(layer_index))
    x_full = np.asarray(all_encoder_layers)[layer]                # (B, S, H)
    tm = np.asarray(token_map).astype(np.int64)                   # (B, T+1)

    pos = np.arange(S)
    seg = np.empty((B, S), dtype=np.int64)
    for b in range(B):
        seg[b] = np.searchsorted(tm[b], pos, side="right") - 1
    valid = pos[None, :] < tm[:, -1:]
    seg = np.where(valid, np.clip(seg, 0, T - 1), T)              # (B, S)
    counts = (tm[:, 1:] - tm[:, :-1]).astype(np.float32)          # (B, T)
    inv = np.zeros((B, S), dtype=np.float32)
    bb = np.arange(B)[:, None]
    iv = seg < T
    inv[iv] = (np.float32(1.0) /
               counts[np.broadcast_to(bb, seg.shape)[iv], seg[iv]])
    inv = inv.astype(NPBF16).astype(np.float32)  # match device bf16 sel

    fcwT = np.ascontiguousarray(np.asarray(fc_w, dtype=np.float32).T)
    if OPT["s2"] == "fp8x2":
        w16 = fcwT * np.float32(16.0)
        w_hi = w16.astype(NPFP8)
        w_lo = (w16 - w_hi.astype(np.float32)).astype(NPFP8)
        w_in = ("fcw8", np.ascontiguousarray(
            np.concatenate([w_hi, w_lo], axis=0)))               # (2H, H) fp8
    else:
        w_in = ("fcwb", np.ascontiguousarray(fcwT.astype(NPBF16)))
    fcb = np.asarray(fc_b, dtype=np.float32).reshape(1, H)
    bias_aux = np.ascontiguousarray(np.concatenate(
        [fcb, np.ones((1, 128), np.float32)], axis=1)).astype(NPBF16)

    x_bf = x_full.astype(NPBF16)                                  # (B, S, H)
    in_maps = []
    for c in range(N_CORES):
        bs = slice(c * B_LOC, (c + 1) * B_LOC)
        # (B_LOC, S) -> (128, NJ) with column j = b*NS + chunk
        seg_t = seg[bs].reshape(NJ, 128).T.astype(np.float32)
        inv_t = inv[bs].reshape(NJ, 128).T
        aux = np.ascontiguousarray(
            np.concatenate([seg_t, inv_t], axis=1))          # (128, 2*NJ)
        in_maps.append({
            "x": np.ascontiguousarray(x_bf[bs].reshape(NJ * 128, H)),
            "selaux": aux,
            "biasaux": bias_aux,
            w_in[0]: w_in[1],
            "identb": np.eye(128, dtype=NPBF16),
        })
    return in_maps


class CachedRunner:
    """Jit/compile/load the bass program once; later calls are pure executes."""

    def __init__(self, nc, donate: bool = True):
        import jax
        from jax.sharding import Mesh, PartitionSpec
        from jax.experimental.shard_map import shard_map
        from concourse import bass2jax

        bass2jax.install_neuronx_cc_hook()
        self.nc = nc
        in_names, out_names, out_avals = [], [], []
        pname = nc.partition_id_tensor.name if nc.partition_id_tensor else None
        for alloc in nc.m.functions[0].allocations:
            if not isinstance(alloc, mybir.MemoryLocationSet):
                continue
            name = alloc.memorylocations[0].name
            if alloc.kind == "ExternalInput":
                if name != pname:
                    in_names.append(name)
            elif alloc.kind == "ExternalOutput":
                shape = tuple(alloc.tensor_shape)
                dtype = mybir.dt.np(alloc.dtype)
                out_names.append(name)
                out_avals.append(jax.core.ShapedArray(shape, dtype))
        self.in_names = list(in_names)
        self.out_names = out_names
        self.out_avals = out_avals
        n_params = len(in_names)
        n_outs = len(out_names)
        all_in_names = list(in_names) + list(out_names)
        if pname is not None:
            all_in_names.append(pname)
        donate_idx = tuple(range(n_params, n_params + n_outs)) if donate else ()

        def _body(*args):
            operands = list(args)
            if pname is not None:
                operands.append(bass2jax.partition_id_tensor())
            outs = bass2jax._bass_exec_p.bind(
                *operands,
                out_avals=tuple(out_avals),
                in_names=tuple(all_in_names),
                out_names=tuple(out_names),
                lowering_input_output_aliases=(),
                sim_require_finite=False,
                sim_require_nnan=False,
                nc=nc,
            )
            return tuple(outs)

        devices = jax.devices()[:N_CORES]
        mesh = Mesh(np.asarray(devices), ("core",))
        in_specs = (PartitionSpec("core"),) * (n_params + n_outs)
        out_specs = (PartitionSpec("core"),) * n_outs
        self.mesh = mesh
        self.sharding = jax.sharding.NamedSharding(mesh, PartitionSpec("core"))
        self.sharded = jax.jit(
            shard_map(_body, mesh=mesh, in_specs=in_specs,
                      out_specs=out_specs, check_rep=False),
            donate_argnums=donate_idx, keep_unused=True)
        self._dev_args = None

    def __call__(self, in_maps):
        concat_in = [
            np.concatenate([np.asarray(in_maps[c][n]) for c in range(N_CORES)], 0)
            for n in self.in_names]
        concat_zeros = [
            np.zeros((N_CORES * a.shape[0], *a.shape[1:]), a.dtype)
            for a in self.out_avals]
        out = self.sharded(*concat_in, *concat_zeros)
        return out  # list of jax arrays, concatenated over cores on axis 0

    def prepare(self, in_maps):
        """device_put all arguments once (requires donate=False runner)."""
        import jax
        concat_in = [
            np.concatenate([np.asarray(in_maps[c][n]) for c in range(N_CORES)], 0)
            for n in self.in_names]
        concat_zeros = [
            np.zeros((N_CORES * a.shape[0], *a.shape[1:]), a.dtype)
            for a in self.out_avals]
        self._dev_args = [jax.device_put(a, self.sharding)
                          for a in concat_in + concat_zeros]
        jax.block_until_ready(self._dev_args)

    def run_prepared(self):
        return self.sharded(*self._dev_args)

    def to_maps(self, out):
        return [
            {n: np.asarray(out[i]).reshape(N_CORES, *self.out_avals[i].shape)[c]
             for i, n in enumerate(self.out_names)}
            for c in range(N_CORES)]


_RUNNER_CACHE: dict = {}


def get_runner(reps: int = 1, loop: bool = False, donate: bool = True,
               bias_mm: bool = False) -> CachedRunner:
    key = (reps, loop, donate, bias_mm, OPT["s2"])
    if key not in _RUNNER_CACHE:
        _RUNNER_CACHE[key] = CachedRunner(
            build_kernel(reps, loop, bias_mm=bias_mm), donate)
    return _RUNNER_CACHE[key]


def kernel(all_encoder_layers, input_mask, token_map, fc_w, fc_b, layer_index):
    in_maps = _host_prep(all_encoder_layers, token_map, fc_w, fc_b, layer_index)
    bias_mm = bool(np.any(np.asarray(fc_b)))
    runner = get_runner(1, bias_mm=bias_mm)
    out = runner(in_maps)
    idx = {n: i for i, n in enumerate(runner.out_names)}
    rep = np.asarray(out[idx["rep"]]).astype(np.float32)
    ote = np.asarray(out[idx["ote"]]).astype(np.float32)
    return rep, ote


# revision 6
# speedup vs baseline: 3.8049x; 3.8049x over previous
"""Trainium2 Bass kernel for CharacterNet segment-mean + FC (segment_reduce).

Reference computation (per batch row b of 32):
  x = all_encoder_layers[layer_index][b]          # (512, 768)
  for t in 0..255: mean_t = mean(x[token_map[b,t]:token_map[b,t+1]])
  ote[b*256+t] = mean_t                           # (8192, 768) output 2
  rep = ote @ fc_w.T + fc_b                       # (8192, 768) output 1

Strategy: data-parallel over batch across 8 NeuronCores (4 rows/core).
All HBM I/O is 16-bit or fp8 (rel-err budget 2e-2 allows it; measured
~4e-3): x/outputs bf16, fc_w as two fp8 tensors (hi + 16x-scaled lo
residual).  The segment mean is a matmul with a one-hot selection
matrix Sel (s, t) built on device; stage 1 computes ote directly in
natural orientation (t on partitions), PE-transposes feed stage 2
(the FC) which runs as fp8 DoubleRow matmuls at 0.5 cyc/row using the
hi/lo split:  rep ~= [m_hi@w_hi + m_hi@w_lo + m_lo@w_hi] / 256, with
m_hi = fp8(16*mean), m_lo = fp8(16*mean - m_hi) (same for w).
"""

import os
import numpy as np
import ml_dtypes

import concourse.bass as bass
import concourse.bacc as bacc
import concourse.mybir as mybir
import concourse.tile as tile
from concourse.bass_utils import run_bass_kernel_spmd  # noqa: F401

N_CORES = 8
B, S, H, T = 32, 512, 768, 256
B_LOC = B // N_CORES          # 4 batch rows per core
NS = S // 128                 # 4 s-chunks per row
NJ = B_LOC * NS               # 16 (128,...) s-chunks per core
NH = H // 128                 # 6 h-chunks

F32 = mybir.dt.float32
BF16 = mybir.dt.bfloat16
FP8 = mybir.dt.float8e4
NPBF16 = ml_dtypes.bfloat16
NPFP8 = ml_dtypes.float8_e4m3

OPT = {
    "s2": os.environ.get("KERNEL_S2", "fp8x2"),  # fp8x2 | bf16
    # NOTE: gpsimd (Pool) cannot access PSUM; psum evicts go DVE/Act only
    "ote_copy": "scalar",    # s1 psum->ote sbuf evict engine
    "rep_copy": "scalar",    # s2 psum->rep sbuf evict (scaled copy)
    "mhi_copy": "vector",    # transpose psum -> m_hi fp8
    "mlo_copy": "vector",    # transpose psum -> m_lo fp8 (stt)
    "mb_copy": "vector",     # bf16-mode transpose evict
    "sel_eng": "vector",
    "in_dma": "sync", "out_dma": "sync",
    "p1": 3, "pt": 2, "p2": 3,
    "w_after": 1,            # emit fc_w DMA after this many x2 pair DMAs
    "x_split_first": True,
}

_ENG = {"vector": "vector", "scalar": "scalar", "gpsimd": "gpsimd"}


def _copy(nc, engine, dst, src_):
    if engine == "scalar":
        nc.scalar.copy(dst, src_)
    elif engine == "gpsimd":
        nc.gpsimd.tensor_copy(dst, src_)
    else:
        nc.vector.tensor_copy(dst, src_)


def _scaled_copy(nc, engine, dst, src_, scale):
    if engine == "scalar":
        nc.scalar.mul(dst, src_, float(scale))
    elif engine == "gpsimd":
        nc.gpsimd.tensor_scalar_mul(dst, src_, float(scale))
    else:
        nc.vector.tensor_scalar_mul(dst, src_, float(scale))


def build_kernel(reps: int = 1, loop: bool = False,
                 bias_mm: bool = False) -> bass.Bass:
    s2_fp8 = OPT["s2"] == "fp8x2"
    nc = bacc.Bacc("TRN2", target_bir_lowering=False, debug=False,
                   num_devices=N_CORES)

    x_d = nc.dram_tensor("x", (NJ * 128, H), BF16, kind="ExternalInput")
    # packed aux: cols 0..15 = seg, 16..31 = inv  (128, 32) f32
    aux_d = nc.dram_tensor("selaux", (128, 2 * NJ), F32, kind="ExternalInput")
    if s2_fp8:
        w_d = nc.dram_tensor("fcw8", (2 * H, H), FP8, kind="ExternalInput")
    else:
        w_d = nc.dram_tensor("fcwb", (H, H), BF16, kind="ExternalInput")
    # packed bias row: [0:H]=fc_b, [H:H+128]=ones  (bf16)
    bias_d = nc.dram_tensor("biasaux", (1, H + 128), BF16,
                            kind="ExternalInput")
    ident_d = nc.dram_tensor("identb", (128, 128), BF16, kind="ExternalInput")
    rep_d = nc.dram_tensor("rep", (B_LOC * T, H), BF16, kind="ExternalOutput")
    ote_d = nc.dram_tensor("ote", (B_LOC * T, H), BF16, kind="ExternalOutput")

    # paired-row-chunk views: [j0][p, q, h] = t[(2*j0+q)*128+p, h]
    x_v = x_d.rearrange("(a q p) h -> a p q h", q=2, p=128)
    rep_v = rep_d.rearrange("(a q p) h -> a p q h", q=2, p=128)
    ote_v = ote_d.rearrange("(a q p) h -> a p q h", q=2, p=128)
    if s2_fp8:
        # (128, 2hl, 6kh, 768n):  w_v[p, l, k, n] = w_d[l*768 + k*128 + p, n]
        w_v = w_d.rearrange("(l k p) n -> p l k n", l=2, p=128)
    else:
        w_v = w_d.rearrange("(k p) n -> p k n", p=128)

    with tile.TileContext(nc) as tc:
        with (
            tc.tile_pool(name="const", bufs=1) as cpool,
            tc.tile_pool(name="xp", bufs=1) as xpool,
            tc.tile_pool(name="selp", bufs=1) as selpool,
            tc.tile_pool(name="mp", bufs=1) as mpool,
            tc.tile_pool(name="wp", bufs=1) as wpool,
            tc.tile_pool(name="ob", bufs=2) as opool,
            tc.tile_pool(name="p1", bufs=OPT["p1"], space="PSUM") as p1pool,
            tc.tile_pool(name="pt", bufs=OPT["pt"], space="PSUM") as ptpool,
            tc.tile_pool(name="p2", bufs=OPT["p2"], space="PSUM") as p2pool,
        ):
            # one-time constants (outside the rep loop)
            iota_t = cpool.tile([128, T], BF16, tag="iota")
            nc.gpsimd.iota(iota_t[:], pattern=[[1, T]], base=0,
                           channel_multiplier=0,
                           allow_small_or_imprecise_dtypes=True)
            ident = cpool.tile([128, 128], BF16, tag="ident")
            nc.sync.dma_start(ident[:], ident_d[:])

            def emit_rep():
                idma = getattr(nc, OPT["in_dma"])
                odma = getattr(nc, OPT["out_dma"])
                aux_sb = cpool.tile([128, 2 * NJ], F32, tag="aux")
                idma.dma_start(aux_sb[:], aux_d[:])
                bias_sb = cpool.tile([1, H + 128], BF16, tag="bias")
                if bias_mm:
                    idma.dma_start(bias_sb[:], bias_d[:])

                # ---- input DMAs + sel builds -------------------------------
                if s2_fp8:
                    w_sb = wpool.tile([128, 2, NH, H], FP8, tag="w")
                else:
                    w_sb = wpool.tile([128, NH, H], BF16, tag="w")
                x2_sb = []

                def emit_w():
                    idma.dma_start(w_sb[:], w_v[:] if not s2_fp8 else w_v)

                for j0 in range(NJ // 2):
                    if j0 == OPT["w_after"]:
                        emit_w()
                    x2 = xpool.tile([128, 2, H], BF16, tag=f"x{j0}")
                    if j0 == 0 and OPT["x_split_first"]:
                        for q in range(2):
                            idma.dma_start(x2[:, q, :],
                                           x_d[q * 128:(q + 1) * 128, :])
                    else:
                        idma.dma_start(x2[:], x_v[j0])
                    x2_sb.append(x2)
                if OPT["w_after"] >= NJ // 2:
                    emit_w()

                sel_eng = getattr(nc, _ENG[OPT["sel_eng"]])
                sel_sb = []
                for j in range(NJ):
                    sel = selpool.tile([128, T], BF16, tag=f"s{j}")
                    # Sel chunk: (s in segment t) * 1/count, bf16
                    sel_eng.tensor_scalar(
                        sel[:], iota_t[:],
                        aux_sb[:, j:j + 1], aux_sb[:, NJ + j:NJ + j + 1],
                        op0=mybir.AluOpType.is_equal,
                        op1=mybir.AluOpType.mult)
                    sel_sb.append(sel)

                # bias broadcast tile (128, H) bf16 via K=1 matmul
                if bias_mm:
                    biasb = cpool.tile([128, H], BF16, tag="biasb")
                    for nh in range(2):
                        pb = p2pool.tile([128, 384], F32, tag="ps2")
                        nc.tensor.matmul(
                            pb[:], bias_sb[:1, H:H + 128],
                            bias_sb[:1, nh * 384:(nh + 1) * 384],
                            start=True, stop=True)
                        _copy(nc, "vector",
                              biasb[:, nh * 384:(nh + 1) * 384], pb[:])

                # ---- per-batch-row stages, software-pipelined on PE --------
                osb = [None] * B_LOC

                def stage1(b):
                    # ote[b] natural layout: psum[t', h] = sum_s Sel[s,t']*x[s,h]
                    osb[b] = opool.tile([128, 2, H], BF16, tag="osb", name="osb")
                    for tq in range(2):
                        for hh in range(2):
                            ps = p1pool.tile([128, 384], F32, tag="ps1")
                            for ks in range(NS):
                                j = b * NS + ks
                                nc.tensor.matmul(
                                    ps[:],
                                    sel_sb[j][:, tq * 128:(tq + 1) * 128],
                                    x2_sb[j // 2][:, j % 2,
                                                  hh * 384:(hh + 1) * 384],
                                    start=(ks == 0), stop=(ks == NS - 1))
                            _copy(nc, OPT["ote_copy"],
                                  osb[b][:, tq, hh * 384:(hh + 1) * 384],
                                  ps[:])
                    odma.dma_start(ote_v[b], osb[b][:])

                mhi = [None] * B_LOC
                mlo = [None] * B_LOC

                def stage_tr(b):
                    # transpose ote[b] -> meanT chunks; evict as fp8 hi/lo
                    # (or single bf16 tile in bf16 mode)
                    if s2_fp8:
                        mhi[b] = mpool.tile([128, NH, T], FP8, tag=f"mh{b}", name=f"mh{b}")
                        mlo[b] = mpool.tile([128, NH, T], FP8, tag=f"ml{b}", name=f"ml{b}")
                    else:
                        mhi[b] = mpool.tile([128, NH, T], BF16, tag=f"mh{b}", name=f"mh{b}")
                    for g in range(3):
                        pt = ptpool.tile([128, 512], BF16, tag="pst")
                        for k in range(4):
                            mh, tq = 2 * g + k // 2, k % 2
                            nc.tensor.transpose(
                                pt[:, k * 128:(k + 1) * 128],
                                osb[b][:, tq, mh * 128:(mh + 1) * 128],
                                ident[:])
                        dhi = mhi[b][:, 2 * g:2 * g + 2, :]
                        if s2_fp8:
                            # m_hi = fp8(16*mean); m_lo = fp8(16*mean - m_hi)
                            _scaled_copy(nc, OPT["mhi_copy"], dhi, pt[:], 16.0)
                            eng = getattr(nc, _ENG[OPT["mlo_copy"]])
                            eng.scalar_tensor_tensor(
                                mlo[b][:, 2 * g:2 * g + 2, :], pt[:], 16.0,
                                dhi,
                                op0=mybir.AluOpType.mult,
                                op1=mybir.AluOpType.subtract)
                        else:
                            _copy(nc, OPT["mb_copy"], dhi, pt[:])

                def stage2(b):
                    rsb = opool.tile([128, 2, H], BF16, tag="rsb")
                    for tq in range(2):
                        for nh in range(2):
                            ps2 = p2pool.tile([128, 384], F32, tag="ps2")
                            nsl = slice(nh * 384, (nh + 1) * 384)
                            tsl = slice(tq * 128, (tq + 1) * 128)
                            if s2_fp8:
                                dr = mybir.MatmulPerfMode.DoubleRow
                                ngrp = 9
                                i = 0
                                for sm, sw in ((0, 0), (0, 1), (1, 0)):
                                    msrc = (mhi, mlo)[sm][b]
                                    for g in range(3):
                                        nc.tensor.matmul(
                                            ps2[:],
                                            msrc[:, 2 * g:2 * g + 2, tsl],
                                            w_sb[:, sw, 2 * g:2 * g + 2, nsl],
                                            start=(i == 0),
                                            stop=(i == ngrp - 1),
                                            perf_mode=dr)
                                        i += 1
                            else:
                                for kh in range(NH):
                                    nc.tensor.matmul(
                                        ps2[:],
                                        mhi[b][:, kh, tsl],
                                        w_sb[:, kh, nsl],
                                        start=(kh == 0), stop=(kh == NH - 1))
                            dst = rsb[:, tq, nsl]
                            scale = (1.0 / 256.0) if s2_fp8 else 1.0
                            if bias_mm:
                                # out = psum*scale + bias
                                nc.vector.scalar_tensor_tensor(
                                    dst, ps2[:], scale, biasb[:, nsl],
                                    op0=mybir.AluOpType.mult,
                                    op1=mybir.AluOpType.add)
                            elif s2_fp8:
                                _scaled_copy(nc, OPT["rep_copy"], dst,
                                             ps2[:], scale)
                            else:
                                _copy(nc, OPT["rep_copy"], dst, ps2[:])
                    odma.dma_start(rep_v[b], rsb[:])

                # PE pipeline: s1(0) s1(1) tr(0) s1(2) s2(0) tr(1) s1(3)
                #              s2(1) tr(2) s2(2) tr(3) s2(3)
                stage1(0)
                stage1(1)
                stage_tr(0)
                stage1(2)
                stage2(0)
                stage_tr(1)
                stage1(3)
                stage2(1)
                stage_tr(2)
                stage2(2)
                stage_tr(3)
                stage2(3)

            if loop and reps > 1:
                with tc.For_i(0, reps, 1,
                              hint_engines=(mybir.EngineType.PE,)):
                    emit_rep()
            else:
                for _ in range(reps):
                    emit_rep()

    nc.compile()
    return nc


def _host_prep(all_encoder_layers, token_map, fc_w, fc_b, layer_index):
    """Slice the chosen layer and build per-core input maps."""
    layer = int(np.asarray(layer_index))
    x_full = np.asarray(all_encoder_layers)[layer]                # (B, S, H)
    tm = np.asarray(token_map).astype(np.int64)                   # (B, T+1)

    pos = np.arange(S)
    seg = np.empty((B, S), dtype=np.int64)
    for b in range(B):
        seg[b] = np.searchsorted(tm[b], pos, side="right") - 1
    valid = pos[None, :] < tm[:, -1:]
    seg = np.where(valid, np.clip(seg, 0, T - 1), T)              # (B, S)
    counts = (tm[:, 1:] - tm[:, :-1]).astype(np.float32)          # (B, T)
    inv = np.zeros((B, S), dtype=np.float32)
    bb = np.arange(B)[:, None]
    iv = seg < T
    inv[iv] = (np.float32(1.0) /
               counts[np.broadcast_to(bb, seg.shape)[iv], seg[iv]])
    inv = inv.astype(NPBF16).astype(np.float32)  # match device bf16 sel

    fcwT = np.ascontiguousarray(np.asarray(fc_w, dtype=np.float32).T)
    if OPT["s2"] == "fp8x2":
        w16 = fcwT * np.float32(16.0)
        w_hi = w16.astype(NPFP8)
        w_lo = (w16 - w_hi.astype(np.float32)).astype(NPFP8)
        w_in = ("fcw8", np.ascontiguousarray(
            np.concatenate([w_hi, w_lo], axis=0)))               # (2H, H) fp8
    else:
        w_in = ("fcwb", np.ascontiguousarray(fcwT.astype(NPBF16)))
    fcb = np.asarray(fc_b, dtype=np.float32).reshape(1, H)
    bias_aux = np.ascontiguousarray(np.concatenate(
        [fcb, np.ones((1, 128), np.float32)], axis=1)).astype(NPBF16)

    x_bf = x_full.astype(NPBF16)                                  # (B, S, H)
    in_maps = []
    for c in range(N_CORES):
        bs = slice(c * B_LOC, (c + 1) * B_LOC)
        # (B_LOC, S) -> (128, NJ) with column j = b*NS + chunk
        seg_t = seg[bs].reshape(NJ, 128).T.astype(np.float32)
        inv_t = inv[bs].reshape(NJ, 128).T
        aux = np.ascontiguousarray(
            np.concatenate([seg_t, inv_t], axis=1))          # (128, 2*NJ)
        in_maps.append({
            "x": np.ascontiguousarray(x_bf[bs].reshape(NJ * 128, H)),
            "selaux": aux,
            "biasaux": bias_aux,
            w_in[0]: w_in[1],
            "identb": np.eye(128, dtype=NPBF16),
        })
    return in_maps


class CachedRunner:
    """Jit/compile/load the bass program once; later calls are pure executes."""

    def __init__(self, nc, donate: bool = True):
        import jax
        from jax.sharding import Mesh, PartitionSpec
        from jax.experimental.shard_map import shard_map
        from concourse import bass2jax

        bass2jax.install_neuronx_cc_hook()
        self.nc = nc
        in_names, out_names, out_avals = [], [], []
        pname = nc.partition_id_tensor.name if nc.partition_id_tensor else None
        for alloc in nc.m.functions[0].allocations:
            if not isinstance(alloc, mybir.MemoryLocationSet):
                continue
            name = alloc.memorylocations[0].name
            if alloc.kind == "ExternalInput":
                if name != pname:
                    in_names.append(name)
            elif alloc.kind == "ExternalOutput":
                shape = tuple(alloc.tensor_shape)
                dtype = mybir.dt.np(alloc.dtype)
                out_names.append(name)
                out_avals.append(jax.core.ShapedArray(shape, dtype))
        self.in_names = list(in_names)
        self.out_names = out_names
        self.out_avals = out_avals
        n_params = len(in_names)
        n_outs = len(out_names)
        all_in_names = list(in_names) + list(out_names)
        if pname is not None:
            all_in_names.append(pname)
        donate_idx = tuple(range(n_params, n_params + n_outs)) if donate else ()

        def _body(*args):
            operands = list(args)
            if pname is not None:
                operands.append(bass2jax.partition_id_tensor())
            outs = bass2jax._bass_exec_p.bind(
                *operands,
                out_avals=tuple(out_avals),
                in_names=tuple(all_in_names),
                out_names=tuple(out_names),
                lowering_input_output_aliases=(),
                sim_require_finite=False,
                sim_require_nnan=False,
                nc=nc,
            )
            return tuple(outs)

        devices = jax.devices()[:N_CORES]
        mesh = Mesh(np.asarray(devices), ("core",))
        in_specs = (PartitionSpec("core"),) * (n_params + n_outs)
        out_specs = (PartitionSpec("core"),) * n_outs
        self.mesh = mesh
        self.sharding = jax.sharding.NamedSharding(mesh, PartitionSpec("core"))
        self.sharded = jax.jit(
            shard_map(_body, mesh=mesh, in_specs=in_specs,
                      out_specs=out_specs, check_rep=False),
            donate_argnums=donate_idx, keep_unused=True)
        self._dev_args = None

    def __call__(self, in_maps):
        concat_in = [
            np.concatenate([np.asarray(in_maps[c][n]) for c in range(N_CORES)], 0)
            for n in self.in_names]
        concat_zeros = [
            np.zeros((N_CORES * a.shape[0], *a.shape[1:]), a.dtype)
            for a in self.out_avals]
        out = self.sharded(*concat_in, *concat_zeros)
        return out  # list of jax arrays, concatenated over cores on axis 0

    def prepare(self, in_maps):
        """device_put all arguments once (requires donate=False runner)."""
        import jax
        concat_in = [
            np.concatenate([np.asarray(in_maps[c][n]) for c in range(N_CORES)], 0)
            for n in self.in_names]
        concat_zeros = [
            np.zeros((N_CORES * a.shape[0], *a.shape[1:]), a.dtype)
            for a in self.out_avals]
        self._dev_args = [jax.device_put(a, self.sharding)
                          for a in concat_in + concat_zeros]
        jax.block_until_ready(self._dev_args)

    def run_prepared(self):
        return self.sharded(*self._dev_args)

    def to_maps(self, out):
        return [
            {n: np.asarray(out[i]).reshape(N_CORES, *self.out_avals[i].shape)[c]
             for i, n in enumerate(self.out_names)}
            for c in range(N_CORES)]


_RUNNER_CACHE: dict = {}


def get_runner(reps: int = 1, loop: bool = False, donate: bool = True,
               bias_mm: bool = False) -> CachedRunner:
    key = (reps, loop, donate, bias_mm, OPT["s2"])
    if key not in _RUNNER_CACHE:
        _RUNNER_CACHE[key] = CachedRunner(
            build_kernel(reps, loop, bias_mm=bias_mm), donate)
    return _RUNNER_CACHE[key]


def kernel(all_encoder_layers, input_mask, token_map, fc_w, fc_b, layer_index):
    in_maps = _host_prep(all_encoder_layers, token_map, fc_w, fc_b, layer_index)
    bias_mm = bool(np.any(np.asarray(fc_b)))
    runner = get_runner(1, bias_mm=bias_mm)
    out = runner(in_maps)
    idx = {n: i for i, n in enumerate(runner.out_names)}
    rep = np.asarray(out[idx["rep"]]).astype(np.float32)
    ote = np.asarray(out[idx["ote"]]).astype(np.float32)
    return rep, ote
